# revision 15
# baseline (speedup 1.0000x reference)
"""Trainium2 Bass kernel for a dense transformer block (pre-LN attention + MLP).

Sharding: 8 cores, pure data/sequence parallel, zero collectives.
Core c handles batch b=c//2 and query-half h=c%2 (1024 query tokens).
Each core redundantly computes K/V for its full batch (2048 tokens), which is
cheaper than a cross-core KV exchange on this chip.  The per-core x shard is
rolled so the core's own 1024 query tokens are always rows 0:1024 (attention
here is permutation-invariant over keys, so rolling keys is harmless).

Host-side folding (numpy):
  ln1 affine -> qkv weights/bias;  1/sqrt(dh) -> q weights/bias
  ls1 -> proj weights/bias;  ln2 affine -> fc1;  ls2 -> fc2
so the device only computes raw (affine-free) layernorms and plain matmuls.

Device dataflow (bf16 matmuls, f32 residual spine):
  LN1 -> PE-transpose -> qT/kT computed feature-major, V token-major with a
  ones column per 64-wide head block (softmax denominators fall out of the
  AV matmul for free); scores computed transposed [k, q] so exp + AV need no
  transposes; softmax division folded into the AV PSUM eviction.
"""

import sys

sys.path.insert(0, "/opt/trn_rl_repo")

from contextlib import ExitStack

import numpy as np
import ml_dtypes

import concourse.bass as bass  # noqa: F401
import concourse.tile as tile
from concourse import bacc, mybir
from concourse.bass_utils import run_bass_kernel_spmd

B, N, D = 4, 2048, 768
H, DH = 12, 64
HID = 4 * D
EPS = 1e-5
P = 128
TKV = 2048  # tokens per core for K/V (full batch)
TQ = 1024  # query tokens per core
NT_KV = TKV // P  # 16
NT_Q = TQ // P  # 8
ND = D // P  # 6
NH = HID // P  # 24
F32 = mybir.dt.float32
BF16 = mybir.dt.bfloat16
OP = mybir.AluOpType
ACTF = mybir.ActivationFunctionType
GELU_FUNC = ACTF.Gelu  # test_sim swaps to Identity (CoreSim lacks Gelu)


def _ln_stats(nc, pool, x_tile, eps_t):
    """mean, rstd of a [128, 768] f32 tile over free dim (768 = 3x256)."""
    v = nc.vector
    sub = 256
    nsub = D // sub
    stats = pool.tile([P, nsub, v.BN_STATS_DIM], F32, tag="stats")
    xg = x_tile.rearrange("p (a s) -> p a s", s=sub)
    for a in range(nsub):
        v.bn_stats(stats[:, a, :], xg[:, a, :])
    mv = pool.tile([P, v.BN_AGGR_DIM], F32, tag="mv")
    v.bn_aggr(mv[:, :], stats[:, :, :])
    rs = pool.tile([P, 1], F32, tag="rs")
    nc.scalar.activation(rs[:, :], mv[:, 1:2], ACTF.Sqrt, bias=eps_t[:, :])
    v.reciprocal(rs[:, :], rs[:, :])
    return mv[:, 0:1], rs


def _ln_transpose(nc, tc, pools, src_tiles, nt, dst, eps_t, ident, tag):
    """LN (no affine) each [128, 768] f32 tile of src, transpose into dst
    [P, ND, nt*128] bf16."""
    v = nc.vector
    stat_pool, lnp, tps = pools
    for ti in range(nt):
        xt = src_tiles(ti)
        mu, rs = _ln_stats(nc, stat_pool, xt, eps_t)
        xn = lnp.tile([P, D], BF16, tag=f"xn{tag}")
        v.tensor_scalar(xn[:, :], xt, mu, rs, op0=OP.subtract, op1=OP.mult)
        for dj in range(ND):
            pst = tps.tile([P, P], BF16, tag=f"t{tag}")
            nc.tensor.transpose(pst[:, :], xn[:, dj * P:(dj + 1) * P], ident[:, :])
            nc.any.tensor_copy(dst[:, dj, ti * P:(ti + 1) * P], pst[:, :])


def build_graph():
    nc = bacc.Bacc("TRN2", target_bir_lowering=False, debug=False, num_devices=8)

    x_ext = nc.declare_dram_parameter("x", [TKV, D], F32, isOutput=False)
    wqkv_ext = nc.declare_dram_parameter("wqkv", [D, 3 * D], BF16, isOutput=False)
    wproj_ext = nc.declare_dram_parameter("wproj", [D, D], BF16, isOutput=False)
    w1_ext = nc.declare_dram_parameter("w1", [D, HID], BF16, isOutput=False)
    w2_ext = nc.declare_dram_parameter("w2", [HID, D], BF16, isOutput=False)
    bqkv_ext = nc.declare_dram_parameter("bqkv", [P, 18], F32, isOutput=False)
    b1_ext = nc.declare_dram_parameter("b1", [P, NH], F32, isOutput=False)
    ident_ext = nc.declare_dram_parameter("ident", [P, P], BF16, isOutput=False)
    out_ext = nc.declare_dram_parameter("out", [TQ, D], F32, isOutput=True)

    with tile.TileContext(nc) as tc:
        emit(nc, tc, x_ext.ap(), out_ext.ap(), wqkv_ext.ap(), wproj_ext.ap(),
             w1_ext.ap(), w2_ext.ap(), bqkv_ext.ap(), b1_ext.ap(), ident_ext.ap())

    nc.compile()
    return nc


def emit(nc, tc, x, out, wqkv_d, wproj_d, w1_d, w2_d, bqkv_d, b1_d, ident_d):
    v = nc.vector
    sc = nc.scalar
    te = nc.tensor

    ctx = ExitStack()
    with ctx:
        # ---------- kernel-lifetime pools ----------
        singles = ctx.enter_context(tc.tile_pool(name="singles", bufs=1))
        stat_pool = ctx.enter_context(tc.tile_pool(name="stat", bufs=4))

        eps_t = singles.tile([P, 1], F32)
        v.memset(eps_t[:, :], EPS)
        ident = singles.tile([P, P], BF16)
        nc.sync.dma_start(ident[:, :], ident_d[:, :])
        bqkv = singles.tile([P, 18], F32)
        nc.sync.dma_start(bqkv[:, :], bqkv_d[:, :])
        b1c = singles.tile([P, NH], F32)
        nc.sync.dma_start(b1c[:, :], b1_d[:, :])

        resid = ctx.enter_context(tc.tile_pool(name="resid", bufs=1))
        x1 = resid.tile([P, NT_Q, D], F32)

        with ExitStack() as attn_ctx:
            xownp = attn_ctx.enter_context(tc.tile_pool(name="xownp", bufs=1))
            x_own = xownp.tile([P, NT_Q, D], F32)  # own tokens, residual spine
            qkvp = attn_ctx.enter_context(tc.tile_pool(name="qkvp", bufs=1))
            qT = qkvp.tile([P, ND, TQ], BF16)
            kT = qkvp.tile([P, ND, TKV], BF16)
            v_sb = qkvp.tile([P, NT_KV, H * (DH + 1)], BF16)

            with ExitStack() as qkv_ctx:
                wqp = qkv_ctx.enter_context(tc.tile_pool(name="wqp", bufs=1))
                wqkv = wqp.tile([P, ND, 3 * D], BF16)
                for dj in range(ND):
                    nc.sync.dma_start(wqkv[:, dj, :], wqkv_d[dj * P:(dj + 1) * P, :])

                xnTp = qkv_ctx.enter_context(tc.tile_pool(name="xnTp", bufs=1))
                xnT = xnTp.tile([P, ND, TKV], BF16)

                # ---- phase A: load x, LN1, transpose ----
                with tc.tile_pool(name="xkv", bufs=2) as xkvp, \
                     tc.tile_pool(name="ln1", bufs=3) as lnp, \
                     tc.tile_pool(name="tps1", bufs=8, space="PSUM") as tps:
                    def src(ti):
                        if ti < NT_Q:
                            nc.sync.dma_start(x_own[:, ti, :],
                                              x[ti * P:(ti + 1) * P, :])
                            return x_own[:, ti, :]
                        t = xkvp.tile([P, D], F32, tag="xkv")
                        nc.sync.dma_start(t[:, :], x[ti * P:(ti + 1) * P, :])
                        return t[:, :]

                    _ln_transpose(nc, tc, (stat_pool, lnp, tps), src, NT_KV,
                                  xnT, eps_t, ident, "1")

                # ---- phase B: QKV matmuls ----
                with tc.tile_pool(name="qkps", bufs=2, space="PSUM") as qps:
                    for fj in range(12):  # 0..6 -> qT, 6..12 -> kT
                        is_q = fj < ND
                        for th in range(1 if is_q else 2):
                            ps = qps.tile([P, 1024], F32, tag="qk")
                            for c in range(2):
                                lo = c * 512
                                for dj in range(ND):
                                    te.matmul(
                                        ps[:, lo:lo + 512],
                                        wqkv[:, dj, fj * P:(fj + 1) * P],
                                        xnT[:, dj, th * 1024 + lo:th * 1024 + lo + 512],
                                        start=(dj == 0), stop=(dj == ND - 1),
                                    )
                            if is_q:
                                v.tensor_scalar(qT[:, fj, :], ps[:, :],
                                                bqkv[:, fj:fj + 1], None, op0=OP.add)
                            else:
                                v.tensor_scalar(
                                    kT[:, fj - ND, th * 1024:(th + 1) * 1024],
                                    ps[:, :], bqkv[:, fj:fj + 1], None, op0=OP.add)

                    # ones columns of v_sb (col 64 of each 65-wide head block)
                    vg = v_sb.rearrange("p a (h c) -> p a h c", h=H)
                    v.memset(vg[:, :, :, DH:DH + 1], 1.0)
                    for ti in range(NT_KV):
                        ps = qps.tile([P, D], F32, tag="v")
                        for lo, ln_ in ((0, 512), (512, 256)):
                            for dj in range(ND):
                                te.matmul(
                                    ps[:, lo:lo + ln_],
                                    xnT[:, dj, ti * P:(ti + 1) * P],
                                    wqkv[:, dj, 2 * D + lo:2 * D + lo + ln_],
                                    start=(dj == 0), stop=(dj == ND - 1),
                                )
                        pg = ps.rearrange("p (h c) -> p h c", h=H)
                        v.tensor_copy(vg[:, ti, :, 0:DH], pg[:, :, :])
            # wqkv + xnT freed here

            # ---- phase C: attention (+ proj weight prefetch) ----
            wpp = attn_ctx.enter_context(tc.tile_pool(name="wpp", bufs=1))
            wproj = wpp.tile([P, ND, D], BF16)
            for dj in range(ND):
                nc.sync.dma_start(wproj[:, dj, :], wproj_d[dj * P:(dj + 1) * P, :])

            attnT = attn_ctx.enter_context(
                tc.tile_pool(name="attnTp", bufs=1)).tile([P, ND, TQ], BF16)

            with tc.tile_pool(name="expp", bufs=4) as expp, \
                 tc.tile_pool(name="sps", bufs=2, space="PSUM") as sps, \
                 tc.tile_pool(name="avps", bufs=2, space="PSUM") as avps, \
                 tc.tile_pool(name="recd", bufs=2, space="DRAM") as recdp, \
                 tc.tile_pool(name="recp", bufs=2) as recp:
                for h in range(H):
                    fj, po = h // 2, (h % 2) * DH
                    av = avps.tile([DH + 1, TQ], F32, tag="av")
                    for kt in range(NT_KV):
                        ps = sps.tile([P, TQ], F32, tag="s")
                        for c in range(2):
                            lo = c * 512
                            te.matmul(
                                ps[:, lo:lo + 512],
                                kT[po:po + DH, fj, kt * P:(kt + 1) * P],
                                qT[po:po + DH, fj, lo:lo + 512],
                                start=True, stop=True,
                            )
                        expS = expp.tile([P, TQ], BF16, tag="e")
                        sc.activation(expS[:, :], ps[:, :], ACTF.Exp)
                        for c in range(2):
                            lo = c * 512
                            te.matmul(
                                av[:, lo:lo + 512],
                                v_sb[:, kt, h * (DH + 1):(h + 1) * (DH + 1)],
                                expS[:, lo:lo + 512],
                                start=(kt == 0), stop=(kt == NT_KV - 1),
                            )
                    rec = recp.tile([1, TQ], F32, tag="r")
                    v.reciprocal(rec[:, :], av[DH:DH + 1, :])
                    recd = recdp.tile([1, TQ], F32, tag="rd")
                    nc.sync.dma_start(recd[:, :], rec[:, :])
                    recb = recp.tile([DH, TQ], F32, tag="rb")
                    nc.sync.dma_start(recb[:, :], recd[0:1, :].to_broadcast((DH, TQ)))
                    v.tensor_tensor(attnT[po:po + DH, fj, :], av[0:DH, :],
                                    recb[:, :], op=OP.mult)
            # expS freed here; qT/kT/v_sb freed at attn_ctx exit

            # ---- phase D: proj + residual ----
            with tc.tile_pool(name="pps", bufs=4, space="PSUM") as pps:
                for ti in range(NT_Q):
                    ps = pps.tile([P, D], F32, tag="p")
                    for lo, ln_ in ((0, 512), (512, 256)):
                        for dj in range(ND):
                            te.matmul(
                                ps[:, lo:lo + ln_],
                                attnT[:, dj, ti * P:(ti + 1) * P],
                                wproj[:, dj, lo:lo + ln_],
                                start=(dj == 0), stop=(dj == ND - 1),
                            )
                    v.tensor_tensor(x1[:, ti, :], ps[:, :], x_own[:, ti, :],
                                    op=OP.add)
        # attnT / wproj / qT / kT / v_sb freed here

        # ---- phase E/F: LN2 + MLP ----
        with ExitStack() as mlp_ctx:
            w12p = mlp_ctx.enter_context(tc.tile_pool(name="w12", bufs=1))
            w1 = w12p.tile([P, ND, HID], BF16)
            for dj in range(ND):
                nc.sync.dma_start(w1[:, dj, :], w1_d[dj * P:(dj + 1) * P, :])
            w2 = w12p.tile([P, NH, D], BF16)
            for fj in range(NH):
                nc.sync.dma_start(w2[:, fj, :], w2_d[fj * P:(fj + 1) * P, :])

            h1T = mlp_ctx.enter_context(
                tc.tile_pool(name="h1Tp", bufs=1)).tile([P, NH, TQ], BF16)

            with ExitStack() as fc1_ctx:
                xn2T = fc1_ctx.enter_context(
                    tc.tile_pool(name="xn2Tp", bufs=1)).tile([P, ND, TQ], BF16)
                with tc.tile_pool(name="ln2", bufs=4) as lnp2, \
                     tc.tile_pool(name="tps2", bufs=8, space="PSUM") as tps2:
                    _ln_transpose(nc, tc, (stat_pool, lnp2, tps2),
                                  lambda ti: x1[:, ti, :], NT_Q, xn2T, eps_t,
                                  ident, "2")

                with tc.tile_pool(name="mps", bufs=4, space="PSUM") as mps:
                    for fj in range(NH):
                        ps = mps.tile([P, TQ], F32, tag="m")
                        for c in range(2):
                            lo = c * 512
                            for dj in range(ND):
                                te.matmul(
                                    ps[:, lo:lo + 512],
                                    w1[:, dj, fj * P:(fj + 1) * P],
                                    xn2T[:, dj, lo:lo + 512],
                                    start=(dj == 0), stop=(dj == ND - 1),
                                )
                        sc.activation(h1T[:, fj, :], ps[:, :], GELU_FUNC,
                                      bias=b1c[:, fj:fj + 1])
            # xn2T freed

            with tc.tile_pool(name="ops", bufs=4, space="PSUM") as ops, \
                 tc.tile_pool(name="outp", bufs=2) as outp:
                for ti in range(NT_Q):
                    ps = ops.tile([P, D], F32, tag="o")
                    for lo, ln_ in ((0, 512), (512, 256)):
                        for fj in range(NH):
                            te.matmul(
                                ps[:, lo:lo + ln_],
                                h1T[:, fj, ti * P:(ti + 1) * P],
                                w2[:, fj, lo:lo + ln_],
                                start=(fj == 0), stop=(fj == NH - 1),
                            )
                    ot = outp.tile([P, D], F32, tag="ot")
                    v.tensor_tensor(ot[:, :], ps[:, :], x1[:, ti, :], op=OP.add)
                    nc.sync.dma_start(out[ti * P:(ti + 1) * P, :], ot[:, :])


def _fold(inputs):
    """Fold LN affines, layer scales, and 1/sqrt(dh) into weights (host numpy)."""
    f = {k: np.asarray(v, dtype=np.float32) for k, v in inputs.items()}
    wqkv = (f["ln1_w"][:, None] * f["qkv_w"]).copy()
    bqkv = (f["qkv_b"] + f["ln1_b"] @ f["qkv_w"]).copy()
    scale = 1.0 / np.sqrt(DH)
    wqkv[:, :D] *= scale
    bqkv[:D] *= scale
    wproj = f["proj_w"] * f["ls1_g"][None, :]
    bproj = f["proj_b"] * f["ls1_g"]
    w1 = f["ln2_w"][:, None] * f["fc1_w"]
    b1 = f["fc1_b"] + f["ln2_b"] @ f["fc1_w"]
    w2 = f["fc2_w"] * f["ls2_g"][None, :]
    b2 = f["fc2_b"] * f["ls2_g"]
    assert np.all(bproj == 0.0) and np.all(b2 == 0.0), (
        "nonzero proj/fc2 bias path not implemented")
    assert np.all(bqkv[2 * D:] == 0.0), "nonzero v bias path not implemented"
    return wqkv, bqkv, wproj, w1, b1, w2


def make_in_maps(inputs):
    x = np.asarray(inputs["x"], dtype=np.float32)
    wqkv, bqkv, wproj, w1, b1, w2 = _fold(inputs)
    bf = ml_dtypes.bfloat16
    common = {
        "wqkv": wqkv.astype(bf),
        "wproj": wproj.astype(bf),
        "w1": w1.astype(bf),
        "w2": w2.astype(bf),
        "bqkv": bqkv.reshape(18, P).T.copy().astype(np.float32),
        "b1": b1.reshape(NH, P).T.copy().astype(np.float32),
        "ident": np.eye(P, dtype=bf),
    }
    in_maps = []
    for c in range(8):
        b, h = c // 2, c % 2
        xb = np.roll(x[b], -h * TQ, axis=0)
        in_maps.append({"x": np.ascontiguousarray(xb), **common})
    return in_maps


_CACHE = {}
TRACE = False


def kernel(**inputs):
    in_maps = make_in_maps(inputs)
    if "nc" not in _CACHE:
        _CACHE["nc"] = build_graph()
    nc = _CACHE["nc"]

    res = run_bass_kernel_spmd(nc, in_maps, core_ids=list(range(8)), trace=TRACE)
    _CACHE["last_result"] = res

    outp = np.empty((B, N, D), dtype=np.float32)
    for c in range(8):
        b, h = c // 2, c % 2
        outp[b, h * TQ:(h + 1) * TQ, :] = res.results[c]["out"]
    return outp
